# revision 2
# baseline (speedup 1.0000x reference)
"""Trainium2 Bass kernel for the vq_codebook classifier problem.

For X [4096, 512] f32, grp [1, 512, 100] f32:
    l1   = sum_d |X[n,d] - g[d,c]|           -> [N, C]
    norm = softmax(-l1, axis=1)
    cs   = (X @ g) / (|X|_row * |g|_col)
    out  = max_c(cs) * softmax(cs, axis=1) * norm

Sharding: data-parallel over N across 8 cores (512 rows each), g replicated.

This environment charges a flat ~50-60us per *instruction* regardless of
size, so the kernel minimizes instruction count (~34 real instructions):

  - l1: per 128-row chunk, ONE in-place tensor_tensor subtract over a
    [128, 100*512] broadcast view (x rows stride-0-repeated over classes,
    f16 g table partition-replicated by a broadcast DMA and destroyed
    in place, re-DMAed per chunk) + ONE segmented
    tensor_reduce(add, apply_absolute_value): 4 x (DMA+sub+reduce).
  - cs: host pre-normalizes rows (xthat = X^T / |X|_row) and columns
    (ghat = g / |g|_col); 4 accumulated matmuls give csT [100, 512]
    directly, then 4 PE transposes return to [n, c] packed [128, 4*100].
  - epilogue on packed [128, (4, 100)] tiles with stride-0 per-row
    broadcasts; both softmax exps write adjacent halves of one tile so a
    single segmented reduce yields all 8 row-sums.
"""

import numpy as np

P = 128
R = 512          # rows per core (4096 / 8)
D = 512
C = 100
CH = R // P      # 4 row chunks
DT = D // P      # 4 d tiles
N_CORES = 8

_CACHE = {}


def _prune_implied_waits(nc):
    """Drop sync waits that are transitively implied.

    If instruction X waits on sem A >= a and sem B >= b, and the producer
    whose update brings B to b itself (directly or transitively) waited on
    A >= a, then X's wait on A is redundant: B reaching b proves A reached
    a. Tile emits such redundant waits liberally; each extra wait costs a
    whole NoOp instruction after _split_excess_waits, and instructions are
    the unit of cost here.

    Soundness: this runtime executes instructions as soon as their sem
    waits are satisfied (no implicit same-queue ordering is assumed), so
    implication must flow ONLY through sem waits: a wait (s >= v) carries
    the transitive closure of the waits held by the instruction whose
    update brings s to v. Per-semaphore updates fire in block order;
    `sem-ge-imm` waits are monotone. Barrier semaphores (eq/dec modes)
    are left untouched.
    """
    def skip(name):
        return name is None or name.startswith("barrier")

    for fn in nc.m.functions:
        for bb in fn.blocks:
            insts = bb.instructions
            # map each monotone sem's k-th cumulative value to its producer
            cum = {}
            producers = {}
            for idx, inst in enumerate(insts):
                si = inst.sync_info
                if si is None:
                    continue
                for u in (si.on_update or []):
                    nm = u.ant_name
                    if skip(nm) or u.update_mode not in ("sem-inc", "sem-add-imm"):
                        continue
                    cum[nm] = cum.get(nm, 0) + u.update_value
                    producers.setdefault(nm, []).append((cum[nm], idx))

            def producer_of(nm, v):
                lst = producers.get(nm)
                if not lst:
                    return None
                lo, hi = 0, len(lst)
                while lo < hi:
                    mid = (lo + hi) // 2
                    if lst[mid][0] >= v:
                        hi = mid
                    else:
                        lo = mid + 1
                return lst[lo][1] if lo < len(lst) else None

            def merge(dst, src):
                for s, v in src.items():
                    if dst.get(s, -1) < v:
                        dst[s] = v

            # C_done[idx]: sem values transitively guaranteed by idx's own
            # waits (NOT by queue position -- this runtime has none).
            # C_sem[nm][k]: closure guaranteed once sem nm has received its
            # first k updates. Since every update is a monotone increment,
            # sem nm >= cum_k proves updates 1..k ALL fired, so the closure
            # unions every producer's knowledge up to k.
            C_done = [None] * len(insts)
            C_sem = {nm: [dict()] for nm in producers}
            nseen = {nm: 0 for nm in producers}

            def closure_of(nm, v):
                lst = producers.get(nm)
                if not lst:
                    return {nm: v}
                lo, hi = 0, len(lst)
                while lo < hi:
                    mid = (lo + hi) // 2
                    if lst[mid][0] >= v:
                        hi = mid
                    else:
                        lo = mid + 1
                if lo >= len(lst):
                    return {nm: v}
                k = lo + 1  # need the first k updates
                chain = C_sem[nm]
                cd = dict(chain[min(k, len(chain) - 1)])
                if cd.get(nm, -1) < v:
                    cd[nm] = v
                return cd

            def merge(dst, src):
                for s2, v2 in src.items():
                    if dst.get(s2, -1) < v2:
                        dst[s2] = v2

            for idx, inst in enumerate(insts):
                si = inst.sync_info
                waits = list(si.on_wait) if (si and si.on_wait) else []
                contribs = []
                for w in waits:
                    nm = w.ant_name
                    if skip(nm) or w.wait_mode != "sem-ge-imm":
                        contribs.append((w, None))
                        continue
                    contribs.append((w, closure_of(nm, w.wait_value)))
                kept = []
                kept_cd = []
                for i_w, (w, cd) in enumerate(contribs):
                    if cd is None:
                        kept.append(w)
                        continue
                    known = {}
                    for cd2 in kept_cd:
                        merge(known, cd2)
                    for j_w in range(i_w + 1, len(contribs)):
                        if contribs[j_w][1] is not None:
                            merge(known, contribs[j_w][1])
                    if known.get(w.ant_name, -1) >= w.wait_value:
                        continue
                    kept.append(w)
                    kept_cd.append(cd)
                if si is not None and len(kept) != len(waits):
                    import concourse.mybir as mb
                    inst.sync_info = mb.SyncInfo(
                        on_wait=kept, on_update=list(si.on_update or [])
                    )
                c_all = {}
                for (_, cd) in contribs:
                    if cd is not None:
                        merge(c_all, cd)
                C_done[idx] = c_all
                # extend C_sem chains for this instruction's updates
                if si is not None:
                    for u in (si.on_update or []):
                        nm = u.ant_name
                        if skip(nm) or u.update_mode not in ("sem-inc", "sem-add-imm"):
                            continue
                        if nm not in C_sem:
                            continue
                        chain = C_sem[nm]
                        prev = chain[-1]
                        nxt = dict(prev)
                        merge(nxt, c_all)
                        cv = producers[nm][nseen[nm]][0]
                        if nxt.get(nm, -1) < cv:
                            nxt[nm] = cv
                        nseen[nm] += 1
                        chain.append(nxt)


def _split_excess_waits(nc, limit=1):
    """walrus in this container rejects instructions carrying more than
    one sync wait ("Too many sync wait commands"). Hoist excess waits
    onto same-engine NoOps inserted immediately before the instruction."""
    import concourse.mybir as mb
    import bass_rust

    n_id = [0]

    def mknop(engine, waits):
        n_id[0] += 1
        return bass_rust.InstNoOp(
            name=f"waitsplit-{n_id[0]}", engine=engine, ins=[], outs=[],
            sync_info=mb.SyncInfo(on_wait=list(waits), on_update=[]),
        )

    for fn in nc.m.functions:
        for bb in fn.blocks:
            insts = bb.instructions
            out = []
            for inst in insts:
                si = inst.sync_info
                if si is not None and si.on_wait and len(si.on_wait) > limit:
                    waits = list(si.on_wait)
                    extra, keep = waits[:-limit], waits[-limit:]
                    for w in extra:
                        out.append(mknop(inst.engine, [w]))
                    inst.sync_info = mb.SyncInfo(
                        on_wait=keep, on_update=list(si.on_update)
                    )
                out.append(inst)
            insts[:] = out


def _build_nc(reps: int = 1):
    import concourse.bass as bass
    import concourse.mybir as mybir
    import concourse.tile as tile
    from concourse.masks import make_identity
    from contextlib import ExitStack

    f32 = mybir.dt.float32
    f16 = mybir.dt.float16
    Alu = mybir.AluOpType
    Act = mybir.ActivationFunctionType
    Ax = mybir.AxisListType

    KW = 2 * D * DT + DT * C          # xr (2048) + xt (2048) + gh (400)

    nc = bass.Bass(target_bir_lowering=False)
    # K packs xr | xt | gh so one DMA loads all three:
    #   K[p, 0:2048]      xr: [p, ch*512 + d] = X[ch*128+p, d]
    #   K[p, 2048:4096]   xt: [p, dt*512 + n] = X[n, dt*128+p]
    #   K[p, 4096:4496]   gh: [p, dt*100 + c] = ghat[dt*128+p, c]
    Kd = nc.declare_dram_parameter("K", [P, KW], f32, isOutput=False)
    # GB[0, c*512 + d] = g[d, c] (f16), partition-broadcast on DMA
    GBd = nc.declare_dram_parameter("GB", [1, C * D], f16, isOutput=False)
    Yd = nc.declare_dram_parameter("Y", [P, CH * C], f32, isOutput=True)

    with ExitStack() as ctx:
        tc = ctx.enter_context(tile.TileContext(nc))
        consts = ctx.enter_context(tc.tile_pool(name="consts", bufs=1))
        kp = ctx.enter_context(tc.tile_pool(name="kp", bufs=2))
        gbp = ctx.enter_context(tc.tile_pool(name="gbp", bufs=1))
        work = ctx.enter_context(tc.tile_pool(name="work", bufs=2))
        ps1p = ctx.enter_context(tc.tile_pool(name="ps1", bufs=2, space="PSUM"))
        ps2p = ctx.enter_context(tc.tile_pool(name="ps2", bufs=2, space="PSUM"))

        ident = consts.tile([P, P], f32, tag="ident")
        make_identity(nc, ident[:])

        # arg1 occupies PSUM cols 12:412 (bank 0), cs cols 412:812 (each
        # 100-col transpose write stays inside one 2KB bank: 412..511 in
        # bank 0, then 512..611 / 612..711 / 712..811 in bank 1), so a
        # single Exp activation covers both softmax arguments [12:812].
        A0 = 12
        C0 = 412

        for _rep in range(reps):
            k = kp.tile([P, KW], f32, tag="k")
            nc.sync.dma_start(k[:], Kd[:, :])
            xr = k[:, 0:D * CH]
            xt = k[:, D * CH:2 * D * CH]
            gh = k[:, 2 * D * CH:KW]

            # ---- l1[n, c] = sum_d |x - g|, packed [128, (4, 100)] ----
            l1t = work.tile([P, CH * C], f32, tag="l1t")
            gb = gbp.tile([P, C * D], f16, tag="gb")
            gv = gb[:, :].rearrange("p (c d) -> p c d", c=C)
            for ch in range(CH):
                nc.sync.dma_start(
                    gb[:], GBd[0:1, :].partition_broadcast(P).squeeze(1)
                )
                xv = xr[:, ch * D:(ch + 1) * D].unsqueeze(1) \
                    .broadcast_to([P, C, D])
                nc.vector.tensor_tensor(gv, xv, gv, Alu.subtract)
                nc.vector.tensor_reduce(
                    l1t[:, ch * C:(ch + 1) * C], gv, Ax.X, Alu.add,
                    apply_absolute_value=True,
                )

            # ---- rxn = 1/|x| per row, packed [128, 4] ----
            # (Ln+Exp live in one activation table set, so no per-rep
            # table switches: rsqrt(x) = exp(-0.5*ln(x)).)
            sq = work.tile([P, CH * D], f32, tag="sq")
            nc.vector.tensor_tensor(sq[:], xr, xr, Alu.mult)
            xn2 = work.tile([P, CH], f32, tag="xn2")
            nc.vector.tensor_reduce(
                xn2[:], sq[:, :].rearrange("p (a d) -> p a d", a=CH),
                Ax.X, Alu.add,
            )
            lnv = work.tile([P, CH], f32, tag="lnv")
            nc.scalar.activation(lnv[:], xn2[:], Act.Ln)
            rxn = work.tile([P, CH], f32, tag="rxn")
            nc.scalar.activation(rxn[:], lnv[:], Act.Exp, scale=-0.5)

            # ---- dotT[c, n] = sum_d ghat[d, c] * x[n, d] ----
            dps = ps1p.tile([C, R], f32, tag="dps")
            for dt in range(DT):
                nc.tensor.matmul(
                    dps[:],
                    lhsT=gh[:, dt * C:(dt + 1) * C],
                    rhs=xt[:, dt * R:(dt + 1) * R],
                    start=(dt == 0),
                    stop=(dt == DT - 1),
                )
            dsb = work.tile([C, R], f32, tag="dsb")
            nc.vector.tensor_copy(dsb[:], dps[:])

            ec = ps2p.tile([P, 1024], f32, tag="ec")
            arg1 = ec[:, A0:A0 + CH * C]
            cs = ec[:, C0:C0 + CH * C]
            # transpose dot to [n, c] packed [128, (4, 100)]
            for ch in range(CH):
                nc.tensor.transpose(
                    ec[:, C0 + ch * C:C0 + (ch + 1) * C],
                    dsb[:, ch * P:(ch + 1) * P],
                    ident[:C, :C],
                )
            cs3 = cs.rearrange("p (a c) -> p a c", a=CH)
            # cs = dot * rxn, in place in PSUM
            nc.vector.tensor_tensor(
                cs3, cs3,
                rxn[:, :].unsqueeze(2).broadcast_to([P, CH, C]),
                Alu.mult,
            )

            # ---- epilogue on packed [128, (4, 100)] tiles ----
            l3 = l1t[:, :].rearrange("p (a c) -> p a c", a=CH)
            m = work.tile([P, CH], f32, tag="m")
            nc.vector.tensor_reduce(m[:], l3, Ax.X, Alu.min)
            # arg1 = m - l1  (<= 0)
            nc.vector.scalar_tensor_tensor(
                arg1.rearrange("p (a c) -> p a c", a=CH),
                l3, -1.0,
                m[:, :].unsqueeze(2).broadcast_to([P, CH, C]),
                Alu.mult, Alu.add,
            )
            conf = work.tile([P, CH], f32, tag="conf")
            nc.vector.tensor_reduce(conf[:], cs3, Ax.X, Alu.max)
            # e1 = exp(arg1), e2 = exp(cs) in ONE activation over [12:812]
            e12 = work.tile([P, 2 * CH * C], f32, tag="e12")
            nc.scalar.activation(e12[:], ec[:, A0:A0 + 2 * CH * C], Act.Exp)
            s12 = work.tile([P, 2 * CH], f32, tag="s12")
            nc.vector.tensor_reduce(
                s12[:], e12[:, :].rearrange("p (a c) -> p a c", a=2 * CH),
                Ax.X, Alu.add,
            )
            den = work.tile([P, CH], f32, tag="den")
            nc.vector.tensor_tensor(
                den[:], s12[:, 0:CH], s12[:, CH:2 * CH], Alu.mult
            )
            rden = work.tile([P, CH], f32, tag="rden")
            nc.vector.reciprocal(rden[:], den[:])
            fac = work.tile([P, CH], f32, tag="fac")
            nc.vector.tensor_tensor(fac[:], conf[:], rden[:], Alu.mult)
            # out = e1 * e2 * fac
            t2 = work.tile([P, CH * C], f32, tag="t2")
            nc.vector.tensor_tensor(
                t2[:, :].rearrange("p (a c) -> p a c", a=CH),
                e12[:, CH * C:2 * CH * C].rearrange("p (a c) -> p a c", a=CH),
                fac[:, :].unsqueeze(2).broadcast_to([P, CH, C]),
                Alu.mult,
            )
            outt = work.tile([P, CH * C], f32, tag="outt")
            nc.vector.tensor_tensor(
                outt[:], e12[:, 0:CH * C], t2[:], Alu.mult
            )
            nc.sync.dma_start(Yd[:, :], outt[:])

    _prune_implied_waits(nc)
    _split_excess_waits(nc)
    return nc


def make_in_maps(X: np.ndarray, grp: np.ndarray) -> list[dict]:
    """Host-side prep: shard X over cores; pure relayout of X plus
    prototype-table (weight) preprocessing for g."""
    X = np.ascontiguousarray(X, dtype=np.float32)
    g2d = np.ascontiguousarray(grp.reshape(D, C), dtype=np.float32)

    ghat = g2d / np.sqrt((g2d * g2d).sum(axis=0, keepdims=True))
    gh_dev = ghat.reshape(DT, P, C).transpose(1, 0, 2).reshape(P, DT * C)
    gb_dev = np.ascontiguousarray(
        g2d.T.reshape(1, C * D).astype(np.float16)
    )

    in_maps = []
    for s in range(N_CORES):
        Xs = X[s * R:(s + 1) * R]                 # [512, 512]
        xr = Xs.reshape(CH, P, D).transpose(1, 0, 2).reshape(P, CH * D)
        xt = Xs.T.reshape(DT, P, R).transpose(1, 0, 2).reshape(P, DT * R)
        K = np.concatenate([xr, xt, gh_dev], axis=1)
        in_maps.append({
            "K": np.ascontiguousarray(K, dtype=np.float32),
            "GB": gb_dev,
        })
    return in_maps


def kernel(X: np.ndarray, grp: np.ndarray) -> np.ndarray:
    from concourse.bass_utils import run_bass_kernel_spmd

    if "nc" not in _CACHE:
        _CACHE["nc"] = _build_nc()
    nc = _CACHE["nc"]

    in_maps = make_in_maps(X, grp)
    last_err = None
    for _attempt in range(3):
        try:
            res = run_bass_kernel_spmd(nc, in_maps, list(range(N_CORES)))
            break
        except Exception as e:  # transient device/tunnel hiccups
            last_err = e
            import time
            time.sleep(2.0)
    else:
        raise last_err
    parts = []
    for i in range(N_CORES):
        y = np.asarray(res.results[i]["Y"])       # [128, 4*100]
        parts.append(y.reshape(P, CH, C).transpose(1, 0, 2).reshape(R, C))
    out = np.concatenate(parts, axis=0)
    return np.ascontiguousarray(out, dtype=np.float32)


# revision 3
# speedup vs baseline: 1.1073x; 1.1073x over previous
"""Trainium2 Bass kernel for the vq_codebook classifier problem.

For X [4096, 512] f32, grp [1, 512, 100] f32:
    l1   = sum_d |X[n,d] - g[d,c]|           -> [N, C]
    norm = softmax(-l1, axis=1)
    cs   = (X @ g) / (|X|_row * |g|_col)
    out  = max_c(cs) * softmax(cs, axis=1) * norm

Sharding: data-parallel over N across 8 cores (512 rows each), g replicated.

This environment charges a flat ~50-60us per *instruction* regardless of
size, so the kernel minimizes instruction count (~34 real instructions):

  - l1: per 128-row chunk, ONE in-place tensor_tensor subtract over a
    [128, 100*512] broadcast view (x rows stride-0-repeated over classes,
    f16 g table partition-replicated by a broadcast DMA and destroyed
    in place, re-DMAed per chunk) + ONE segmented
    tensor_reduce(add, apply_absolute_value): 4 x (DMA+sub+reduce).
  - cs: host pre-normalizes rows (xthat = X^T / |X|_row) and columns
    (ghat = g / |g|_col); 4 accumulated matmuls give csT [100, 512]
    directly, then 4 PE transposes return to [n, c] packed [128, 4*100].
  - epilogue on packed [128, (4, 100)] tiles with stride-0 per-row
    broadcasts; both softmax exps write adjacent halves of one tile so a
    single segmented reduce yields all 8 row-sums.
"""

import numpy as np

P = 128
R = 512          # rows per core (4096 / 8)
D = 512
C = 100
CH = R // P      # 4 row chunks
DT = D // P      # 4 d tiles
N_CORES = 8

_CACHE = {}


def _prune_implied_waits(nc):
    """Drop sync waits that are transitively implied.

    If instruction X waits on sem A >= a and sem B >= b, and the producer
    whose update brings B to b itself (directly or transitively) waited on
    A >= a, then X's wait on A is redundant: B reaching b proves A reached
    a. Tile emits such redundant waits liberally; each extra wait costs a
    whole NoOp instruction after _split_excess_waits, and instructions are
    the unit of cost here.

    Soundness: this runtime executes instructions as soon as their sem
    waits are satisfied (no implicit same-queue ordering is assumed), so
    implication must flow ONLY through sem waits: a wait (s >= v) carries
    the transitive closure of the waits held by the instruction whose
    update brings s to v. Per-semaphore updates fire in block order;
    `sem-ge-imm` waits are monotone. Barrier semaphores (eq/dec modes)
    are left untouched.
    """
    def skip(name):
        return name is None or name.startswith("barrier")

    for fn in nc.m.functions:
        for bb in fn.blocks:
            insts = bb.instructions
            # map each monotone sem's k-th cumulative value to its producer
            cum = {}
            producers = {}
            for idx, inst in enumerate(insts):
                si = inst.sync_info
                if si is None:
                    continue
                for u in (si.on_update or []):
                    nm = u.ant_name
                    if skip(nm) or u.update_mode not in ("sem-inc", "sem-add-imm"):
                        continue
                    cum[nm] = cum.get(nm, 0) + u.update_value
                    producers.setdefault(nm, []).append((cum[nm], idx))

            def producer_of(nm, v):
                lst = producers.get(nm)
                if not lst:
                    return None
                lo, hi = 0, len(lst)
                while lo < hi:
                    mid = (lo + hi) // 2
                    if lst[mid][0] >= v:
                        hi = mid
                    else:
                        lo = mid + 1
                return lst[lo][1] if lo < len(lst) else None

            def merge(dst, src):
                for s, v in src.items():
                    if dst.get(s, -1) < v:
                        dst[s] = v

            # C_done[idx]: sem values transitively guaranteed by idx's own
            # waits (NOT by queue position -- this runtime has none).
            # C_sem[nm][k]: closure guaranteed once sem nm has received its
            # first k updates. Since every update is a monotone increment,
            # sem nm >= cum_k proves updates 1..k ALL fired, so the closure
            # unions every producer's knowledge up to k.
            C_done = [None] * len(insts)
            C_sem = {nm: [dict()] for nm in producers}
            nseen = {nm: 0 for nm in producers}

            def closure_of(nm, v):
                lst = producers.get(nm)
                if not lst:
                    return {nm: v}
                lo, hi = 0, len(lst)
                while lo < hi:
                    mid = (lo + hi) // 2
                    if lst[mid][0] >= v:
                        hi = mid
                    else:
                        lo = mid + 1
                if lo >= len(lst):
                    return {nm: v}
                k = lo + 1  # need the first k updates
                chain = C_sem[nm]
                cd = dict(chain[min(k, len(chain) - 1)])
                if cd.get(nm, -1) < v:
                    cd[nm] = v
                return cd

            def merge(dst, src):
                for s2, v2 in src.items():
                    if dst.get(s2, -1) < v2:
                        dst[s2] = v2

            for idx, inst in enumerate(insts):
                si = inst.sync_info
                waits = list(si.on_wait) if (si and si.on_wait) else []
                contribs = []
                for w in waits:
                    nm = w.ant_name
                    if skip(nm) or w.wait_mode != "sem-ge-imm":
                        contribs.append((w, None))
                        continue
                    contribs.append((w, closure_of(nm, w.wait_value)))
                kept = []
                kept_cd = []
                for i_w, (w, cd) in enumerate(contribs):
                    if cd is None:
                        kept.append(w)
                        continue
                    known = {}
                    for cd2 in kept_cd:
                        merge(known, cd2)
                    for j_w in range(i_w + 1, len(contribs)):
                        if contribs[j_w][1] is not None:
                            merge(known, contribs[j_w][1])
                    if known.get(w.ant_name, -1) >= w.wait_value:
                        continue
                    kept.append(w)
                    kept_cd.append(cd)
                if si is not None and len(kept) != len(waits):
                    import concourse.mybir as mb
                    inst.sync_info = mb.SyncInfo(
                        on_wait=kept, on_update=list(si.on_update or [])
                    )
                c_all = {}
                for (_, cd) in contribs:
                    if cd is not None:
                        merge(c_all, cd)
                C_done[idx] = c_all
                # extend C_sem chains for this instruction's updates
                if si is not None:
                    for u in (si.on_update or []):
                        nm = u.ant_name
                        if skip(nm) or u.update_mode not in ("sem-inc", "sem-add-imm"):
                            continue
                        if nm not in C_sem:
                            continue
                        chain = C_sem[nm]
                        prev = chain[-1]
                        nxt = dict(prev)
                        merge(nxt, c_all)
                        cv = producers[nm][nseen[nm]][0]
                        if nxt.get(nm, -1) < cv:
                            nxt[nm] = cv
                        nseen[nm] += 1
                        chain.append(nxt)


def _split_excess_waits(nc, limit=1):
    """walrus in this container rejects instructions carrying more than
    one sync wait ("Too many sync wait commands"). Hoist excess waits
    onto same-engine NoOps inserted immediately before the instruction."""
    import concourse.mybir as mb
    import bass_rust

    n_id = [0]

    def mknop(engine, waits):
        n_id[0] += 1
        return bass_rust.InstNoOp(
            name=f"waitsplit-{n_id[0]}", engine=engine, ins=[], outs=[],
            sync_info=mb.SyncInfo(on_wait=list(waits), on_update=[]),
        )

    for fn in nc.m.functions:
        for bb in fn.blocks:
            insts = bb.instructions
            out = []
            for inst in insts:
                si = inst.sync_info
                if si is not None and si.on_wait and len(si.on_wait) > limit:
                    waits = list(si.on_wait)
                    extra, keep = waits[:-limit], waits[-limit:]
                    for w in extra:
                        out.append(mknop(inst.engine, [w]))
                    inst.sync_info = mb.SyncInfo(
                        on_wait=keep, on_update=list(si.on_update)
                    )
                out.append(inst)
            insts[:] = out


def _build_nc(reps: int = 1):
    import concourse.bass as bass
    import concourse.mybir as mybir
    import concourse.tile as tile
    from concourse.masks import make_identity
    from contextlib import ExitStack

    f32 = mybir.dt.float32
    f16 = mybir.dt.float16
    Alu = mybir.AluOpType
    Act = mybir.ActivationFunctionType
    Ax = mybir.AxisListType

    KW = 2 * D * DT + DT * C          # xr (2048) + xt (2048) + gh (400)

    nc = bass.Bass(target_bir_lowering=False)
    # K packs xr | xt | gh so one DMA loads all three:
    #   K[p, 0:2048]      xr: [p, ch*512 + d] = X[ch*128+p, d]
    #   K[p, 2048:4096]   xt: [p, dt*512 + n] = X[n, dt*128+p]
    #   K[p, 4096:4496]   gh: [p, dt*100 + c] = ghat[dt*128+p, c]
    Kd = nc.declare_dram_parameter("K", [P, KW], f32, isOutput=False)
    # GB[0, c*512 + d] = g[d, c] (f16), partition-broadcast on DMA
    GBd = nc.declare_dram_parameter("GB", [1, C * D], f16, isOutput=False)
    Yd = nc.declare_dram_parameter("Y", [P, CH * C], f32, isOutput=True)

    with ExitStack() as ctx:
        tc = ctx.enter_context(tile.TileContext(nc))
        consts = ctx.enter_context(tc.tile_pool(name="consts", bufs=1))
        kp = ctx.enter_context(tc.tile_pool(name="kp", bufs=2))
        gbp = ctx.enter_context(tc.tile_pool(name="gbp", bufs=1))
        work = ctx.enter_context(tc.tile_pool(name="work", bufs=2))
        ps1p = ctx.enter_context(tc.tile_pool(name="ps1", bufs=2, space="PSUM"))
        ps2p = ctx.enter_context(tc.tile_pool(name="ps2", bufs=2, space="PSUM"))

        ident = consts.tile([P, P], f32, tag="ident")
        make_identity(nc, ident[:])

        for _rep in range(reps):
            k = kp.tile([P, KW], f32, tag="k")
            nc.sync.dma_start(k[:], Kd[:, :])
            xr = k[:, 0:D * CH]
            xt = k[:, D * CH:2 * D * CH]
            gh = k[:, 2 * D * CH:KW]

            # ---- l1[n, c] = sum_d |x - g|, packed [128, (4, 100)] ----
            # lc packs [-l1 | cs]: one max-reduce then yields both the
            # softmin shift (-m) and the confidence max.
            lc = work.tile([P, 2 * CH * C], f32, tag="lc")
            nl1 = lc[:, 0:CH * C]
            cs = lc[:, CH * C:2 * CH * C]
            gb = gbp.tile([P, C * D], f16, tag="gb")
            gv = gb[:, :].rearrange("p (c d) -> p c d", c=C)
            for ch in range(CH):
                nc.sync.dma_start(
                    gb[:], GBd[0:1, :].partition_broadcast(P).squeeze(1)
                )
                xv = xr[:, ch * D:(ch + 1) * D].unsqueeze(1) \
                    .broadcast_to([P, C, D])
                nc.vector.tensor_tensor(gv, xv, gv, Alu.subtract)
                nc.vector.tensor_reduce(
                    nl1[:, ch * C:(ch + 1) * C], gv, Ax.X, Alu.add,
                    apply_absolute_value=True, negate=True,
                )

            # ---- rxn = 1/|x| per row, packed [128, 4] ----
            # (Ln+Exp live in one activation table set, so no per-rep
            # table switches: rsqrt(x) = exp(-0.5*ln(x)).)
            sq = work.tile([P, CH * D], f32, tag="sq")
            nc.vector.tensor_tensor(sq[:], xr, xr, Alu.mult)
            xn2 = work.tile([P, CH], f32, tag="xn2")
            nc.vector.tensor_reduce(
                xn2[:], sq[:, :].rearrange("p (a d) -> p a d", a=CH),
                Ax.X, Alu.add,
            )
            lnv = work.tile([P, CH], f32, tag="lnv")
            nc.scalar.activation(lnv[:], xn2[:], Act.Ln)
            rxn = work.tile([P, CH], f32, tag="rxn")
            nc.scalar.activation(rxn[:], lnv[:], Act.Exp, scale=-0.5)

            # ---- dotT[c, n] = sum_d ghat[d, c] * x[n, d] ----
            dps = ps1p.tile([C, R], f32, tag="dps")
            for dt in range(DT):
                nc.tensor.matmul(
                    dps[:],
                    lhsT=gh[:, dt * C:(dt + 1) * C],
                    rhs=xt[:, dt * R:(dt + 1) * R],
                    start=(dt == 0),
                    stop=(dt == DT - 1),
                )
            dsb = work.tile([C, R], f32, tag="dsb")
            nc.vector.tensor_copy(dsb[:], dps[:])
            # transpose dot to [n, c] packed [128, (4, 100)]
            dot = ps2p.tile([P, CH * C], f32, tag="dot")
            for ch in range(CH):
                nc.tensor.transpose(
                    dot[:, ch * C:(ch + 1) * C],
                    dsb[:, ch * P:(ch + 1) * P],
                    ident[:C, :C],
                )
            # cs = dot * rxn, into the right half of lc
            cs3 = cs.rearrange("p (a c) -> p a c", a=CH)
            nc.vector.tensor_tensor(
                cs3, dot[:, :].rearrange("p (a c) -> p a c", a=CH),
                rxn[:, :].unsqueeze(2).broadcast_to([P, CH, C]),
                Alu.mult,
            )

            # ---- epilogue on packed [128, (4, 100)] tiles ----
            # one reduce: max(-l1) = -m and max(cs) = conf, [128, 8]
            mc = work.tile([P, 2 * CH], f32, tag="mc")
            nc.vector.tensor_reduce(
                mc[:], lc[:, :].rearrange("p (a c) -> p a c", a=2 * CH),
                Ax.X, Alu.max,
            )
            nm = mc[:, 0:CH]
            conf = mc[:, CH:2 * CH]
            # arg1 = m - l1 = (-l1) - (-m), in place over nl1  (<= 0)
            nl3 = nl1.rearrange("p (a c) -> p a c", a=CH)
            nc.vector.scalar_tensor_tensor(
                nl3, nl3, 1.0,
                nm.unsqueeze(2).broadcast_to([P, CH, C]),
                Alu.mult, Alu.subtract,
            )
            # e1 = exp(arg1), e2 = exp(cs) in ONE activation over lc
            e12 = work.tile([P, 2 * CH * C], f32, tag="e12")
            nc.scalar.activation(e12[:], lc[:], Act.Exp)
            s12 = work.tile([P, 2 * CH], f32, tag="s12")
            nc.vector.tensor_reduce(
                s12[:], e12[:, :].rearrange("p (a c) -> p a c", a=2 * CH),
                Ax.X, Alu.add,
            )
            den = work.tile([P, CH], f32, tag="den")
            nc.vector.tensor_tensor(
                den[:], s12[:, 0:CH], s12[:, CH:2 * CH], Alu.mult
            )
            rden = work.tile([P, CH], f32, tag="rden")
            nc.vector.reciprocal(rden[:], den[:])
            fac = work.tile([P, CH], f32, tag="fac")
            nc.vector.tensor_tensor(fac[:], conf[:, :], rden[:], Alu.mult)
            # out = e1 * e2 * fac
            t2 = work.tile([P, CH * C], f32, tag="t2")
            nc.vector.tensor_tensor(
                t2[:, :].rearrange("p (a c) -> p a c", a=CH),
                e12[:, CH * C:2 * CH * C].rearrange("p (a c) -> p a c", a=CH),
                fac[:, :].unsqueeze(2).broadcast_to([P, CH, C]),
                Alu.mult,
            )
            outt = work.tile([P, CH * C], f32, tag="outt")
            nc.vector.tensor_tensor(
                outt[:], e12[:, 0:CH * C], t2[:], Alu.mult
            )
            nc.sync.dma_start(Yd[:, :], outt[:])

    _prune_implied_waits(nc)
    _split_excess_waits(nc)
    return nc


def make_in_maps(X: np.ndarray, grp: np.ndarray) -> list[dict]:
    """Host-side prep: shard X over cores; pure relayout of X plus
    prototype-table (weight) preprocessing for g."""
    X = np.ascontiguousarray(X, dtype=np.float32)
    g2d = np.ascontiguousarray(grp.reshape(D, C), dtype=np.float32)

    ghat = g2d / np.sqrt((g2d * g2d).sum(axis=0, keepdims=True))
    gh_dev = ghat.reshape(DT, P, C).transpose(1, 0, 2).reshape(P, DT * C)
    gb_dev = np.ascontiguousarray(
        g2d.T.reshape(1, C * D).astype(np.float16)
    )

    in_maps = []
    for s in range(N_CORES):
        Xs = X[s * R:(s + 1) * R]                 # [512, 512]
        xr = Xs.reshape(CH, P, D).transpose(1, 0, 2).reshape(P, CH * D)
        xt = Xs.T.reshape(DT, P, R).transpose(1, 0, 2).reshape(P, DT * R)
        K = np.concatenate([xr, xt, gh_dev], axis=1)
        in_maps.append({
            "K": np.ascontiguousarray(K, dtype=np.float32),
            "GB": gb_dev,
        })
    return in_maps


def kernel(X: np.ndarray, grp: np.ndarray) -> np.ndarray:
    from concourse.bass_utils import run_bass_kernel_spmd

    if "nc" not in _CACHE:
        _CACHE["nc"] = _build_nc()
    nc = _CACHE["nc"]

    in_maps = make_in_maps(X, grp)
    last_err = None
    for _attempt in range(3):
        try:
            res = run_bass_kernel_spmd(nc, in_maps, list(range(N_CORES)))
            break
        except Exception as e:  # transient device/tunnel hiccups
            last_err = e
            import time
            time.sleep(2.0)
    else:
        raise last_err
    parts = []
    for i in range(N_CORES):
        y = np.asarray(res.results[i]["Y"])       # [128, 4*100]
        parts.append(y.reshape(P, CH, C).transpose(1, 0, 2).reshape(R, C))
    out = np.concatenate(parts, axis=0)
    return np.ascontiguousarray(out, dtype=np.float32)
